# revision 1
# baseline (speedup 1.0000x reference)
"""Trainium2 Bass kernel for CausalWanSelfAttention (block-causal attention with
RMSNorm'd+RoPE'd q/k), distributed over 8 NeuronCores via SPMD.

Sharding:
  - Token quarters (tiles 4/4/4/5 of 128): cores 0-3 compute the q projection
    (full 1536 dims) for their quarter, cores 4-7 the k projection. Every core
    also computes a [quarter x 768] block of v. RMSNorm and RoPE are local
    (token-major layout).
  - Exchange #1 is an AllToAll: each producer routes, per destination core,
    only that core's 2 head-slots (q/k rows d-major via PE transposes, v
    columns). Consumers then hold [head, all-tokens] q/k/v for their slots.
  - Attention: 16 head-slots (2 full causal triangles per core; the 4
    duplicate slots on cores 4-7 are discarded). Scores are computed
    transposed (sT[keys,q] = kT.T @ qT) so P^T feeds the V-matmul directly;
    the softmax denominator comes from a ones-vector matmul; no running max
    (scores are O(1) after RMS norm).
  - Exchange #2 is an AllToAll of (oT, l) sliced by destination token block;
    each core runs the output projection for a [544-token x 768-out-dim]
    block; the host assembles the blocks.
All matmuls run in float32r (full-rate fp32 on the PE at free-dim>=256).
"""
import math
import sys

sys.path.insert(0, "/opt/trn_rl_repo")

import numpy as np

import concourse.bass as bass
import concourse.tile as tile
from concourse import bacc, mybir
from concourse.bass import ds
from concourse.masks import make_identity

F32 = mybir.dt.float32
F32R = mybir.dt.float32r
I32 = mybir.dt.int32
AF = mybir.ActivationFunctionType
ALU = mybir.AluOpType

# problem constants (hardcoded per contract)
P = 128
DIM = 1536
NH = 12
HD = 128
S = 2080
SPAD = 2176
NTL = 17
F_, H_, W_ = 4, 20, 26
EPS = 1e-6
N_CORES = 8

QTILES = [4, 4, 4, 5]          # token tiles per quarter
QSTART = [0, 512, 1024, 1536]  # token start per quarter
QLEN = [512, 512, 512, 640]
TQ = 640                       # uniform (padded) tokens per core
NT = 5                         # uniform token tiles per core
VHALF = 768

# A2A #1 shard layout per destination: [2 slots x 128 qk rows, 640 tokens]
# followed by [640 tokens, 2 slots x 128 v cols]
QK1 = 2 * P * TQ               # 163840
V1 = TQ * 2 * P                # 163840
SHARD1 = QK1 + V1              # 327680 floats (1.31 MB)
V1_ROWS = SHARD1 // 256        # rows of 256 in the flat [.., 256] view
V1_ROW0 = QK1 // 256           # v-part starts at row 640 within a shard

TOK_BLK = 544                  # O-proj tokens per core
NHALF = 768                    # O-proj out-dims per core
OT_ROWS = 129                  # 128 oT rows + 1 l row
SHARD2 = 2 * OT_ROWS * TOK_BLK
GROUPS = [(0, 3), (4, 7), (8, 11), (12, 16)]
SCALE = 1.0 / math.sqrt(HD)

_CACHED_NC = None


def _slot_head(c, slot):
    return c if slot == 0 else 8 + (c % 4)


def _head_dests(h):
    """Destination (core, slot) pairs that attend head h."""
    if h < 8:
        return [(h, 0)]
    return [(h - 8, 1), (h - 4, 1)]


def _chunks(total, step=512):
    out = []
    a = 0
    while a < total:
        out.append((a, min(step, total - a)))
        a += step
    return out


def _bank_chunks(off, n, bank=512):
    """Split [off, off+n) at absolute multiples of `bank` (PSUM bank size)."""
    out = []
    a = off
    end = off + n
    while a < end:
        b = min(end, (a // bank + 1) * bank)
        out.append((a, b - a))
        a = b
    return out


def build_nc():
    nc = bacc.Bacc("TRN2", target_bir_lowering=False, debug=False,
                   num_devices=N_CORES)

    x_my = nc.dram_tensor("x_my", [TQ, DIM], F32, kind="ExternalInput").ap()
    w_proj = nc.dram_tensor("w_proj", [DIM, DIM], F32, kind="ExternalInput").ap()
    wv_half = nc.dram_tensor("wv_half", [DIM, VHALF], F32, kind="ExternalInput").ap()
    wo_slice = nc.dram_tensor("wo_slice", [DIM, NHALF], F32, kind="ExternalInput").ap()
    ang_in = nc.dram_tensor("ang_in", [TQ, 128], F32, kind="ExternalInput").ap()
    tabs = nc.dram_tensor("tabs", [1, 32], I32, kind="ExternalInput").ap()
    outT = nc.dram_tensor("outT", [NHALF, TOK_BLK], F32, kind="ExternalOutput").ap()

    with tile.TileContext(nc) as tc:
        _body(tc, x_my, w_proj, wv_half, wo_slice, ang_in, tabs, outT)
    nc.compile()
    return nc


def _body(tc, *args):
    from contextlib import ExitStack
    with ExitStack() as es:
        const = es.enter_context(tc.tile_pool(name="const", bufs=1))
        dram = es.enter_context(tc.tile_pool(name="dram", bufs=1, space="DRAM"))
        shared = es.enter_context(tc.tile_pool(name="shared", bufs=1, space="DRAM"))
        _body2(tc, const, dram, shared, *args)


def _body2(tc, const, dram, shared,
           x_my, w_proj, wv_half, wo_slice, ang_in, tabs, outT):
    nc = tc.nc

    ident = const.tile([P, P], F32)
    make_identity(nc, ident)
    ones_f32 = const.tile([P, 1], F32)
    nc.vector.memset(ones_f32, 1.0)
    ones_col = const.tile([P, 1], F32R)
    nc.vector.tensor_copy(ones_col, ones_f32)
    eps_t = const.tile([P, 1], F32)
    nc.vector.memset(eps_t, EPS)
    tab_sb = const.tile([1, 32], I32)
    nc.sync.dma_start(out=tab_sb, in_=tabs)

    # rope tables: ang_in = [sin-angles | cos-angles], both reduced to [-pi,pi)
    cos_sb = const.tile([P, NT, 64], F32)
    sin_sb = const.tile([P, NT, 64], F32)
    ang_sb = const.tile([P, NT, 128], F32)
    nc.sync.dma_start(out=ang_sb, in_=ang_in.rearrange("(a p) c -> p a c", p=P))
    nc.scalar.activation(sin_sb[:, :, :], ang_sb[:, :, 0:64], AF.Sin)
    nc.scalar.activation(cos_sb[:, :, :], ang_sb[:, :, 64:128], AF.Sin)

    # collective buffers
    send1 = dram.tile([N_CORES, SHARD1], F32)
    recv1 = dram.tile([N_CORES, SHARD1], F32)
    send2 = dram.tile([N_CORES, 2, OT_ROWS, TOK_BLK], F32)
    recv2 = dram.tile([N_CORES, 2 * OT_ROWS * TOK_BLK], F32)
    rl_dram = dram.tile([NH, TOK_BLK], F32)

    # views of send1
    s1_qk = [send1[d:d + 1, 0:QK1].rearrange("one (r t) -> (one r) t", t=TQ)
             for d in range(N_CORES)]
    s1_v = [send1[d:d + 1, QK1:SHARD1].rearrange("one (t v) -> (one t) v", v=256)
            for d in range(N_CORES)]

    # ---------------- Phase A+B+C: xT, q/k projection + RMS + rope, v ----------------
    with tc.tile_pool(name="resident", bufs=1) as res, \
         tc.tile_pool(name="xtiles", bufs=2) as xtiles, \
         tc.tile_pool(name="wpool", bufs=2) as wpool, \
         tc.tile_pool(name="work", bufs=4) as work, \
         tc.tile_pool(name="evict", bufs=4) as evict, \
         tc.tile_pool(name="psA", bufs=3, space="PSUM") as psA, \
         tc.tile_pool(name="psT", bufs=2, space="PSUM") as psT:

        xT = res.tile([P, 12, TQ], F32R)          # x^T, d-major (3.9MB)
        q_raw = res.tile([P, NT, DIM], F32)       # projection out, token-major

        # A: load x tiles, PE-transpose into xT
        for t in range(NT):
            x_t = xtiles.tile([P, DIM], F32, tag="x_t")
            nc.sync.dma_start(out=x_t, in_=x_my[t * P:(t + 1) * P, :])
            for k in range(12):
                tp = psT.tile([P, P], F32, tag="tp")
                nc.tensor.transpose(tp, x_t[:, k * P:(k + 1) * P], ident)
                nc.vector.tensor_copy(xT[:, k, t * P:(t + 1) * P], tp)

        # B: q (or k) projection, n-chunk outer so weights stream once
        ssq = work.tile([P, NT, 3], F32, tag="ssq", bufs=1)
        for n in range(3):
            w_n = wpool.tile([P, 12, 512], F32R, tag="w_n")
            nc.sync.dma_start(
                out=w_n,
                in_=w_proj[:, n * 512:(n + 1) * 512]
                .rearrange("(k p) d -> p k d", p=P).bitcast(F32R))
            for t in range(NT):
                mm_ps = psA.tile([P, 512], F32, tag="mm")
                for k in range(12):
                    nc.tensor.matmul(mm_ps, xT[:, k, t * P:(t + 1) * P],
                                     w_n[:, k, :], start=(k == 0), stop=(k == 11))
                sq_scr = work.tile([P, 512], F32, tag="sq_scr")
                nc.scalar.activation(sq_scr, mm_ps, AF.Square,
                                     accum_out=ssq[:, t, n:n + 1])
                nc.vector.tensor_copy(q_raw[:, t, n * 512:(n + 1) * 512], mm_ps)

        # RMS + rope per token tile (in-place on q_raw)
        for t in range(NT):
            s01 = work.tile([P, 1], F32, tag="s01")
            nc.vector.tensor_tensor(s01, ssq[:, t, 0:1], ssq[:, t, 1:2], ALU.add)
            stot = work.tile([P, 1], F32, tag="stot")
            nc.vector.tensor_tensor(stot, s01, ssq[:, t, 2:3], ALU.add)
            sq_t = work.tile([P, 1], F32, tag="sq_t")
            nc.scalar.activation(sq_t, stot, AF.Sqrt, bias=eps_t,
                                 scale=1.0 / DIM)
            rsq = work.tile([P, 1], F32, tag="rsq")
            nc.vector.reciprocal(rsq, sq_t)
            crsq = work.tile([P, 64], F32, tag="crsq")
            srsq = work.tile([P, 64], F32, tag="srsq")
            nc.vector.tensor_scalar_mul(crsq, cos_sb[:, t, :], rsq)
            nc.vector.tensor_scalar_mul(srsq, sin_sb[:, t, :], rsq)
            cb = bass.AP(tensor=crsq.tensor, offset=crsq.offset,
                         ap=[crsq.ap[0], [0, NH], crsq.ap[1]])
            sbb = bass.AP(tensor=srsq.tensor, offset=srsq.offset,
                          ap=[srsq.ap[0], [0, NH], srsq.ap[1]])
            qh = q_raw[:, t, :].rearrange("p (h c two) -> p h c two", h=NH, two=2)
            qe = qh[:, :, :, 0]
            qo = qh[:, :, :, 1]
            tA = work.tile([P, NH, 64], F32, tag="tA")
            tB = work.tile([P, NH, 64], F32, tag="tB")
            tC = work.tile([P, NH, 64], F32, tag="tC")
            tD = work.tile([P, NH, 64], F32, tag="tD")
            nc.vector.tensor_tensor(tA, qe, cb, ALU.mult)
            nc.vector.tensor_tensor(tB, qo, sbb, ALU.mult)
            nc.vector.tensor_tensor(tC, qe, sbb, ALU.mult)
            nc.vector.tensor_tensor(tD, qo, cb, ALU.mult)
            nc.vector.tensor_tensor(qe, tA, tB, ALU.subtract)
            nc.vector.tensor_tensor(qo, tC, tD, ALU.add)

        # transpose roped q/k into a per-head stage, then one DMA per route
        for h in range(12):
            stage_h = evict.tile([P, TQ], F32, tag="stage_h", bufs=3)
            for t in range(NT):
                tp = psT.tile([P, P], F32, tag="tp")
                nc.tensor.transpose(tp, q_raw[:, t, h * P:(h + 1) * P], ident)
                nc.vector.tensor_copy(stage_h[:, t * P:(t + 1) * P], tp)
            for (d, sl) in _head_dests(h):
                nc.sync.dma_start(
                    out=s1_qk[d][sl * P:(sl + 1) * P, :], in_=stage_h)

        # C: v projection [TQ, VHALF] into v_sb, then route columns per dest
        v_sb = res.tile([P, NT, VHALF], F32)
        for n0, nn in ((0, 512), (512, 256)):
            wv_n = wpool.tile([P, 12, 512], F32R, tag="w_n")
            nc.sync.dma_start(
                out=wv_n[:, :, 0:nn],
                in_=wv_half[:, n0:n0 + nn]
                .rearrange("(k p) d -> p k d", p=P).bitcast(F32R))
            for t in range(NT):
                mm_ps = psA.tile([P, 512], F32, tag="mm")
                for k in range(12):
                    nc.tensor.matmul(mm_ps[:, 0:nn], xT[:, k, t * P:(t + 1) * P],
                                     wv_n[:, k, 0:nn], start=(k == 0), stop=(k == 11))
                nc.vector.tensor_copy(v_sb[:, t, n0:n0 + nn], mm_ps[:, 0:nn])

        # v routing: local head column lh is head lh on a q-core and head
        # 6+lh on a k-core. Write BOTH halves' destination patterns
        # statically - a consumer only reads the shards of the 4 ranks of the
        # correct half for each slot, so wrong-half writes are dead data.
        for lh in range(6):
            dests = set(_head_dests(lh)) | set(_head_dests(6 + lh))
            for (d, sl) in sorted(dests):
                nc.sync.dma_start(
                    out=s1_v[d].rearrange("(a p) v -> p a v", p=P)
                    [:, :, sl * P:(sl + 1) * P],
                    in_=v_sb[:, :, lh * P:(lh + 1) * P])

    # ---------------- A2A #1 ----------------
    nc.gpsimd.collective_compute(
        "AllToAll", ALU.bypass, replica_groups=[list(range(N_CORES))],
        ins=[send1.opt()], outs=[recv1.opt()])

    r1_qk = [recv1[r:r + 1, 0:QK1].rearrange("one (w t) -> (one w) t", t=TQ)
             for r in range(N_CORES)]
    r1_v2d = recv1.rearrange("r c -> (r c)").rearrange("(a v) -> a v", v=256)

    # ---------------- Phase D: attention, 2 head slots ----------------
    with tc.tile_pool(name="attn", bufs=1) as attn, \
         tc.tile_pool(name="ptp", bufs=3) as ptp, \
         tc.tile_pool(name="aev", bufs=2) as aev, \
         tc.tile_pool(name="psO", bufs=1, space="PSUM") as psO, \
         tc.tile_pool(name="psS", bufs=2, space="PSUM") as psS, \
         tc.tile_pool(name="psL", bufs=1, space="PSUM") as psL:
        for slot in range(2):
            qTc = attn.tile([P, NTL * P], F32R, tag="qTc", bufs=2)
            kTc = attn.tile([P, NTL * P], F32R, tag="kTc", bufs=2)
            Vc = attn.tile([P, NTL, P], F32R, tag="Vc", bufs=2)

            for r in range(4):
                tb = 4 * r * P
                nl = QTILES[r] * P
                nc.sync.dma_start(
                    out=qTc[:, tb:tb + nl],
                    in_=r1_qk[r][slot * P:(slot + 1) * P, 0:nl].bitcast(F32R))
                nc.sync.dma_start(
                    out=kTc[:, tb:tb + nl],
                    in_=r1_qk[r + 4][slot * P:(slot + 1) * P, 0:nl].bitcast(F32R))
            with nc.gpsimd.register(f"vr_{slot}") as rr:
                for r in range(4):
                    idx = slot * 4 + r
                    nc.gpsimd.reg_load(rr, tab_sb[0:1, idx:idx + 1])
                    vrow = nc.gpsimd.snap(rr)
                    nc.gpsimd.dma_start(
                        out=Vc[:, 4 * r:4 * r + QTILES[r], :],
                        in_=r1_v2d[ds(vrow, QTILES[r] * P),
                                   slot * P:(slot + 1) * P]
                        .rearrange("(a p) d -> p a d", p=P).bitcast(F32R))

            for (t0, t1) in GROUPS:
                ng = (t1 - t0 + 1) * P
                oT_ps = psO.tile([P, ng], F32, tag="oT")
                l_ps = psL.tile([1, ng], F32, tag="l")
                for kt in range(t1 + 1):
                    c0 = max(t0, kt)
                    off = (c0 - t0) * P
                    n = (t1 - c0 + 1) * P
                    sT_ps = psS.tile([P, n], F32, tag="sT")
                    for (ja, jn) in _chunks(n):
                        nc.tensor.matmul(sT_ps[:, ja:ja + jn],
                                         kTc[:, kt * P:(kt + 1) * P],
                                         qTc[:, c0 * P + ja:c0 * P + ja + jn],
                                         start=True, stop=True)
                    PT = ptp.tile([P, n], F32R, tag="PT")
                    nc.scalar.activation(PT, sT_ps, AF.Exp, scale=SCALE)
                    if kt == 16:
                        # zero pad-key rows 32..128 (memset can't write f32r;
                        # a base partition of 32 may span at most 32 rows)
                        nc.vector.tensor_scalar_mul(PT[32:64, :], PT[32:64, :], 0.0)
                        nc.vector.tensor_scalar_mul(PT[64:P, :], PT[64:P, :], 0.0)
                    # accumulation groups are per PSUM bank: a bank's last
                    # write happens at kt == its highest column tile
                    for (ja, jn) in _bank_chunks(off, n):
                        bank = ja // 512
                        fin = (kt == min(t1, t0 + 4 * bank + 3))
                        nc.tensor.matmul(oT_ps[:, ja:ja + jn],
                                         Vc[:, kt, :], PT[:, ja - off:ja - off + jn],
                                         start=(kt == 0), stop=fin)
                        nc.tensor.matmul(l_ps[:, ja:ja + jn],
                                         ones_col, PT[:, ja - off:ja - off + jn],
                                         start=(kt == 0), stop=fin)
                # evict group results, slicing into destination token blocks
                oT_sb = aev.tile([P, ng], F32, tag="oT_sb")
                nc.vector.tensor_copy(oT_sb, oT_ps)
                l_sb = aev.tile([1, ng], F32, tag="l_sb")
                nc.vector.tensor_copy(l_sb, l_ps)
                g0 = t0 * P
                for j in range(4):
                    a = max(g0, j * TOK_BLK)
                    b = min(g0 + ng, (j + 1) * TOK_BLK)
                    if a >= b:
                        continue
                    for dd in (j, j + 4):
                        nc.sync.dma_start(
                            out=send2[dd, slot, 0:P, a - j * TOK_BLK:b - j * TOK_BLK],
                            in_=oT_sb[:, a - g0:b - g0])
                        nc.sync.dma_start(
                            out=send2[dd, slot, P:P + 1,
                                      a - j * TOK_BLK:b - j * TOK_BLK],
                            in_=l_sb[:, a - g0:b - g0])

    # ---------------- A2A #2 ----------------
    nc.gpsimd.collective_compute(
        "AllToAll", ALU.bypass, replica_groups=[list(range(N_CORES))],
        ins=[send2.opt()], outs=[recv2.opt()])

    r2 = recv2.rearrange("r (s o t) -> r s o t", s=2, o=OT_ROWS)

    def head_src(h):
        return (h, 0) if h < 8 else (h - 8, 1)

    # ---------------- Phase E: output projection ----------------
    with tc.tile_pool(name="oproj", bufs=1) as op, \
         tc.tile_pool(name="owork", bufs=3) as ow, \
         tc.tile_pool(name="psP", bufs=2, space="PSUM") as psP:

        wo_sb = op.tile([P, 12, NHALF], F32R)
        nc.sync.dma_start(out=wo_sb,
                          in_=wo_slice.rearrange("(k p) d -> p k d", p=P)
                          .bitcast(F32R))

        l_all = op.tile([NH, TOK_BLK], F32)
        oT_asm = op.tile([P, NH, TOK_BLK], F32R)

        for h in range(NH):
            rk, sl = head_src(h)
            nc.sync.dma_start(out=l_all[h:h + 1, :],
                              in_=r2[rk, sl, P:P + 1, :])
        rl = op.tile([NH, TOK_BLK], F32)
        nc.vector.reciprocal(rl, l_all)
        nc.sync.dma_start(out=rl_dram, in_=rl)

        for h in range(NH):
            rk, sl = head_src(h)
            oTh = ow.tile([P, TOK_BLK], F32, tag="oTh")
            nc.sync.dma_start(out=oTh, in_=r2[rk, sl, 0:P, :])
            rlb = ow.tile([P, TOK_BLK], F32, tag="rlb")
            rl_bc = bass.AP(tensor=rl_dram.tensor,
                            offset=rl_dram.offset + h * TOK_BLK,
                            ap=[[0, P], [1, TOK_BLK]])
            nc.sync.dma_start(out=rlb, in_=rl_bc)
            nc.vector.tensor_tensor(oT_asm[:, h, :], oTh, rlb, ALU.mult)

        for m in range(6):
            ps = psP.tile([P, TOK_BLK], F32, tag="psP")
            for (ja, jn) in _chunks(TOK_BLK):
                for k in range(NH):
                    nc.tensor.matmul(ps[:, ja:ja + jn],
                                     wo_sb[:, k, m * P:(m + 1) * P],
                                     oT_asm[:, k, ja:ja + jn],
                                     start=(k == 0), stop=(k == NH - 1))
            oev = ow.tile([P, TOK_BLK], F32, tag="oev")
            nc.vector.tensor_copy(oev, ps)
            nc.sync.dma_start(out=outT[m * P:(m + 1) * P, :], in_=oev)


# ======================= host side =======================

def _expected_mask():
    blk = np.arange(SPAD) // P
    return (blk[:, None] >= blk[None, :]) & (np.arange(SPAD)[None, :] < S)


def _host_prep(x, freqs, wq, wk, wv, wo):
    """Build the 8 per-core input maps."""
    x_pad = np.zeros((SPAD, DIM), np.float32)
    x_pad[:S] = x[0]

    # rope angle table (pure gather from freqs)
    t = np.arange(S)
    fi = t // (H_ * W_)
    hi = (t % (H_ * W_)) // W_
    wi = t % W_
    ang = np.zeros((SPAD, 64), np.float32)
    ang[:S, 0:22] = freqs[fi, 0:22]
    ang[:S, 22:43] = freqs[hi, 22:43]
    ang[:S, 43:64] = freqs[wi, 43:64]

    in_maps = []
    for c in range(N_CORES):
        qr = c % 4
        x_my = np.zeros((TQ, DIM), np.float32)
        x_my[:QLEN[qr]] = x_pad[QSTART[qr]:QSTART[qr] + QLEN[qr]]
        ang_q = ang[QSTART[qr]:QSTART[qr] + QLEN[qr]]
        ang_my = np.zeros((TQ, 128), np.float32)
        twopi = 2.0 * math.pi
        # reduce into [-pi, pi): the ACT Sin LUT only covers |x| < 4
        ang_my[:QLEN[qr], 0:64] = np.mod(ang_q + math.pi, twopi) - math.pi
        ang_my[:QLEN[qr], 64:128] = np.mod(ang_q + math.pi / 2.0 + math.pi,
                                           twopi) - math.pi

        tabs = np.zeros((1, 32), np.int32)
        # consumer: v source rank row offsets per slot, per quarter
        for slot in range(2):
            head = _slot_head(c, slot)
            vbase = 0 if head < 6 else 4
            for r in range(4):
                tabs[0, slot * 4 + r] = (vbase + r) * V1_ROWS + V1_ROW0
        # producer: v column routing, 6 local heads x up to 2 dests
        half = 0 if c < 4 else 1
        for lh in range(6):
            h = half * 6 + lh
            dests = _head_dests(h)
            for j in range(2):
                base = 8 + 2 * (lh * 2 + j)
                if j < len(dests):
                    d, sl = dests[j]
                    tabs[0, base] = d * V1_ROWS + V1_ROW0
                    tabs[0, base + 1] = sl * P
                else:
                    tabs[0, base] = N_CORES * V1_ROWS + V1_ROW0  # dummy shard
                    tabs[0, base + 1] = 0

        in_maps.append({
            "x_my": x_my,
            "w_proj": np.ascontiguousarray(wq if c < 4 else wk),
            "wv_half": np.ascontiguousarray(
                wv[:, :VHALF] if c < 4 else wv[:, VHALF:]),
            "wo_slice": np.ascontiguousarray(
                wo[:, (c // 4) * NHALF:(c // 4 + 1) * NHALF]),
            "ang_in": ang_my,
            "tabs": tabs,
        })
    return in_maps


def get_nc():
    global _CACHED_NC
    if _CACHED_NC is None:
        _CACHED_NC = build_nc()
    return _CACHED_NC


def kernel(x, freqs, wq, bq, wk, bk, wv, bv, wo, bo, gq, gk,
           seq_lens, grid_sizes, mask, _run=None):
    x = np.asarray(x, np.float32)
    freqs = np.asarray(freqs, np.float32)
    wq, wk, wv, wo = (np.asarray(w, np.float32) for w in (wq, wk, wv, wo))

    assert x.shape == (1, S, DIM)
    assert int(np.asarray(seq_lens)[0]) == S, "teacher-forcing path not implemented"
    assert tuple(np.asarray(grid_sizes)[0]) == (F_, H_, W_)
    for b in (bq, bk, bv, bo):
        assert not np.any(np.asarray(b)), "nonzero bias not supported"
    for g in (gq, gk):
        assert np.all(np.asarray(g) == 1.0), "non-unit norm gain not supported"
    assert np.array_equal(np.asarray(mask), _expected_mask()), "unexpected mask"

    in_maps = _host_prep(x, freqs, wq, wk, wv, wo)

    if _run is None:
        from concourse.bass_utils import run_bass_kernel_spmd
        nc = get_nc()
        res = run_bass_kernel_spmd(nc, in_maps, list(range(N_CORES)))
        outs = [res.results[c]["outT"] for c in range(N_CORES)]
    else:
        outs = _run(in_maps)

    out = np.zeros((SPAD, DIM), np.float32)
    for c in range(N_CORES):
        r0 = (c % 4) * TOK_BLK
        n0 = (c // 4) * NHALF
        out[r0:r0 + TOK_BLK, n0:n0 + NHALF] = outs[c].T
    return out[:S][None]



# revision 9
# speedup vs baseline: 1.4839x; 1.4839x over previous
"""Trainium2 Bass kernel for CausalWanSelfAttention (block-causal attention with
RMSNorm'd+RoPE'd q/k), distributed over 8 NeuronCores via SPMD.

v2: fp16 datapath + static routing + pipelined collectives.

Sharding:
  - Token quarters (tiles 4/4/4/5 of 128): cores 0-3 compute the q projection
    (full 1536 dims) for their quarter + the LEFT v half (heads 0-5); cores
    4-7 compute k + the RIGHT v half (heads 6-11). x arrives pre-transposed
    (host) so no on-device transposes are needed for the projections.
  - Head slots: slot L of core d = head L[d] in [0..5], slot R = head
    L[d]+6. Heads 2,3,8,9 are computed twice (cores 6,7 duplicate; their
    output is discarded) which keeps the SPMD control flow uniform while
    making every A2A route static.
  - Collectives (all fp16): A2A_v (v columns per dest slot-head), A2A_qk
    split into L-rows and R-rows so slot-L attention can start one collective
    earlier, A2A_o per slot (oT + softmax denominator l per token block).
  - Attention: scores computed transposed (sT[keys,q] = kT.T @ qT) so P^T
    feeds the V-matmul directly; softmax denominator via ones-vector matmul;
    no running max (exp is offset by -4 to keep fp16 in range; the offset
    cancels in o/l).
  - Output projection: per-core [544-token x 768-out-dim] block, accumulated
    in two passes (L heads under the in-flight A2A_oR, then R heads).
"""
import math
import sys

sys.path.insert(0, "/opt/trn_rl_repo")

import numpy as np

import concourse.bass as bass
import concourse.tile as tile
from concourse import bacc, mybir
from concourse.masks import make_identity

F32 = mybir.dt.float32
F16 = mybir.dt.float16
AF = mybir.ActivationFunctionType
ALU = mybir.AluOpType

# problem constants (hardcoded per contract)
P = 128
DIM = 1536
NH = 12
HD = 128
S = 2080
SPAD = 2176
NTL = 17
F_, H_, W_ = 4, 20, 26
EPS = 1e-6
N_CORES = 8

QTILES = [4, 4, 4, 5]          # token tiles per quarter
QSTART = [0, 512, 1024, 1536]  # token start per quarter
QLEN = [512, 512, 512, 640]
TQ = 640                       # uniform (padded) tokens per core
NT = 5                         # uniform token tiles per core
VHALF = 768

# head-slot tables: slot L of core d = LTAB[d], slot R = LTAB[d] + 6.
LTAB = [0, 1, 2, 3, 4, 5, 2, 3]
# source core for head h at the o-projection (dup heads use the low core)
#   h < 6: core h's... head h is slot L of core d where LTAB[d]==h (lowest d)
HSRC = {h: LTAB.index(h) for h in range(6)}

TOK_BLK = 544                  # O-proj tokens per core
NHALF = 768                    # O-proj out-dims per core
OT_ROWS = 129                  # 128 oT rows + 1 l row
GROUPS = [(0, 3), (4, 7), (8, 11), (12, 16)]
SCALE = 1.0 / math.sqrt(HD)
EXP_OFF = -4.0                 # exp offset; cancels in o/l, keeps fp16 finite

_CACHED_NC = None


def _chunks(total, step=512):
    out = []
    a = 0
    while a < total:
        out.append((a, min(step, total - a)))
        a += step
    return out


def _bank_chunks(off, n, bank=512):
    """Split [off, off+n) at absolute multiples of `bank` (PSUM bank size)."""
    out = []
    a = off
    end = off + n
    while a < end:
        b = min(end, (a // bank + 1) * bank)
        out.append((a, b - a))
        a = b
    return out


def build_nc():
    nc = bacc.Bacc("TRN2", target_bir_lowering=False, debug=False,
                   num_devices=N_CORES)

    xT_my = nc.dram_tensor("xT_my", [DIM, TQ], F16, kind="ExternalInput").ap()
    w_qk = nc.dram_tensor("w_qk", [DIM, DIM], F16, kind="ExternalInput").ap()
    wv_half = nc.dram_tensor("wv_half", [DIM, VHALF], F16,
                             kind="ExternalInput").ap()
    wo_slice = nc.dram_tensor("wo_slice", [DIM, NHALF], F16,
                              kind="ExternalInput").ap()
    cos_in = nc.dram_tensor("cos_in", [TQ, 64], F16, kind="ExternalInput").ap()
    sin_in = nc.dram_tensor("sin_in", [TQ, 64], F16, kind="ExternalInput").ap()
    outT = nc.dram_tensor("outT", [NHALF, TOK_BLK], F32,
                          kind="ExternalOutput").ap()

    with tile.TileContext(nc) as tc:
        _body(tc, xT_my, w_qk, wv_half, wo_slice, cos_in, sin_in, outT)
    nc.compile()
    return nc


def _body(tc, *args):
    from contextlib import ExitStack
    with ExitStack() as es:
        const = es.enter_context(tc.tile_pool(name="const", bufs=1))
        dram = es.enter_context(tc.tile_pool(name="dram", bufs=1, space="DRAM"))
        _body2(tc, const, dram, *args)


def _body2(tc, const, dram,
           xT_my, w_qk, wv_half, wo_slice, cos_in, sin_in, outT):
    nc = tc.nc

    ident = const.tile([P, P], F16)
    make_identity(nc, ident)
    ones_col = const.tile([P, 1], F16)
    nc.vector.memset(ones_col, 1.0)
    eps_t = const.tile([P, 1], F32)
    nc.vector.memset(eps_t, EPS)
    neg4 = const.tile([P, 1], F32)
    nc.vector.memset(neg4, EXP_OFF)

    cos_sb = const.tile([P, NT, 64], F16)
    sin_sb = const.tile([P, NT, 64], F16)
    nc.sync.dma_start(out=cos_sb, in_=cos_in.rearrange("(a p) c -> p a c", p=P))
    nc.sync.dma_start(out=sin_sb, in_=sin_in.rearrange("(a p) c -> p a c", p=P))

    # wo lives until phase E -> allocate in the whole-kernel pool
    wo_sb = const.tile([P, 12, NHALF], F16)
    nc.sync.dma_start(out=wo_sb,
                      in_=wo_slice.rearrange("(k p) d -> p k d", p=P))

    # collective buffers (all fp16)
    send_v = dram.tile([N_CORES, TQ * P], F16)
    recv_v = dram.tile([N_CORES, TQ * P], F16)
    send_qa = dram.tile([N_CORES, P, TQ], F16)
    recv_qa = dram.tile([N_CORES, P, TQ], F16)
    send_qb = dram.tile([N_CORES, P, TQ], F16)
    recv_qb = dram.tile([N_CORES, P, TQ], F16)
    send_oa = dram.tile([N_CORES, OT_ROWS, TOK_BLK], F16)
    recv_oa = dram.tile([N_CORES, OT_ROWS, TOK_BLK], F16)
    send_ob = dram.tile([N_CORES, OT_ROWS, TOK_BLK], F16)
    recv_ob = dram.tile([N_CORES, OT_ROWS, TOK_BLK], F16)
    rl_dram = dram.tile([NH, TOK_BLK], F16)

    # ------------- resident tensors (weights + x^T), prefetched -------------
    with tc.tile_pool(name="res", bufs=1) as res, \
         tc.tile_pool(name="work", bufs=4) as work, \
         tc.tile_pool(name="psA", bufs=3, space="PSUM") as psA, \
         tc.tile_pool(name="psT", bufs=2, space="PSUM") as psT:

        xT = res.tile([P, 12, TQ], F16)
        nc.sync.dma_start(out=xT,
                          in_=xT_my.rearrange("(k p) t -> p k t", p=P))
        wv_sb = res.tile([P, 12, VHALF], F16)
        nc.sync.dma_start(out=wv_sb,
                          in_=wv_half.rearrange("(k p) d -> p k d", p=P))
        wqk_sb = res.tile([P, 12, DIM], F16)
        nc.sync.dma_start(out=wqk_sb,
                          in_=w_qk.rearrange("(k p) d -> p k d", p=P))

        # ---------------- Phase V: v projection + routing ----------------
        v_sb = res.tile([P, NT, VHALF], F16)
        for t in range(NT):
            for (n0, nn) in ((0, 512), (512, 256)):
                mm_ps = psA.tile([P, 512], F32, tag="mm")
                for k in range(12):
                    nc.tensor.matmul(mm_ps[:, 0:nn], xT[:, k, t * P:(t + 1) * P],
                                     wv_sb[:, k, n0:n0 + nn],
                                     start=(k == 0), stop=(k == 11))
                nc.vector.tensor_copy(v_sb[:, t, n0:n0 + nn], mm_ps[:, 0:nn])
        for d in range(N_CORES):
            c = LTAB[d]
            nc.sync.dma_start(
                out=send_v[d:d + 1, :].rearrange("one (a p v) -> (one p) a v",
                                                 p=P, v=P),
                in_=v_sb[:, :, c * P:(c + 1) * P])
        nc.gpsimd.collective_compute(
            "AllToAll", ALU.bypass, replica_groups=[list(range(N_CORES))],
            ins=[send_v.opt()], outs=[recv_v.opt()])

        # ---------------- Phase QK: projection + RMS + rope ----------------
        q_raw = res.tile([P, NT, DIM], F16)
        ssq = work.tile([P, NT, 3], F32, tag="ssq", bufs=1)
        for t in range(NT):
            for n in range(3):
                mm_ps = psA.tile([P, 512], F32, tag="mm")
                for k in range(12):
                    nc.tensor.matmul(mm_ps, xT[:, k, t * P:(t + 1) * P],
                                     wqk_sb[:, k, n * 512:(n + 1) * 512],
                                     start=(k == 0), stop=(k == 11))
                sq_scr = work.tile([P, 512], F16, tag="sq_scr")
                nc.scalar.activation(sq_scr, mm_ps, AF.Square,
                                     accum_out=ssq[:, t, n:n + 1])
                nc.vector.tensor_copy(q_raw[:, t, n * 512:(n + 1) * 512], mm_ps)

            # RMS + rope for tile t (in-place on q_raw)
            s01 = work.tile([P, 1], F32, tag="s01")
            nc.vector.tensor_tensor(s01, ssq[:, t, 0:1], ssq[:, t, 1:2], ALU.add)
            stot = work.tile([P, 1], F32, tag="stot")
            nc.vector.tensor_tensor(stot, s01, ssq[:, t, 2:3], ALU.add)
            sq_t = work.tile([P, 1], F32, tag="sq_t")
            nc.scalar.activation(sq_t, stot, AF.Sqrt, bias=eps_t,
                                 scale=1.0 / DIM)
            rsq = work.tile([P, 1], F32, tag="rsq")
            nc.vector.reciprocal(rsq, sq_t)
            crsq = work.tile([P, 64], F16, tag="crsq")
            srsq = work.tile([P, 64], F16, tag="srsq")
            nc.vector.tensor_scalar_mul(crsq, cos_sb[:, t, :], rsq)
            nc.vector.tensor_scalar_mul(srsq, sin_sb[:, t, :], rsq)
            cb = bass.AP(tensor=crsq.tensor, offset=crsq.offset,
                         ap=[crsq.ap[0], [0, NH], crsq.ap[1]])
            sbb = bass.AP(tensor=srsq.tensor, offset=srsq.offset,
                          ap=[srsq.ap[0], [0, NH], srsq.ap[1]])
            qh = q_raw[:, t, :].rearrange("p (h c two) -> p h c two", h=NH, two=2)
            qe = qh[:, :, :, 0]
            qo = qh[:, :, :, 1]
            tA = work.tile([P, NH, 64], F16, tag="tA")
            tB = work.tile([P, NH, 64], F16, tag="tB")
            tC = work.tile([P, NH, 64], F16, tag="tC")
            tD = work.tile([P, NH, 64], F16, tag="tD")
            nc.vector.tensor_tensor(tA, qe, cb, ALU.mult)
            nc.vector.tensor_tensor(tB, qo, sbb, ALU.mult)
            nc.vector.tensor_tensor(tC, qe, sbb, ALU.mult)
            nc.vector.tensor_tensor(tD, qo, cb, ALU.mult)
            nc.vector.tensor_tensor(qe, tA, tB, ALU.subtract)
            nc.vector.tensor_tensor(qo, tC, tD, ALU.add)

        # transpose roped q/k to d-major and route L / R head rows
        qT_all = res.tile([P, 12, TQ], F16)
        for h in range(12):
            for t in range(NT):
                tp = psT.tile([P, P], F16, tag="tp")
                nc.tensor.transpose(tp, q_raw[:, t, h * P:(h + 1) * P], ident)
                nc.vector.tensor_copy(qT_all[:, h, t * P:(t + 1) * P], tp)
        for d in range(N_CORES):
            nc.sync.dma_start(out=send_qa[d], in_=qT_all[:, LTAB[d], :])
            nc.sync.dma_start(out=send_qb[d], in_=qT_all[:, LTAB[d] + 6, :])
        nc.gpsimd.collective_compute(
            "AllToAll", ALU.bypass, replica_groups=[list(range(N_CORES))],
            ins=[send_qa.opt()], outs=[recv_qa.opt()])
        nc.gpsimd.collective_compute(
            "AllToAll", ALU.bypass, replica_groups=[list(range(N_CORES))],
            ins=[send_qb.opt()], outs=[recv_qb.opt()])

    # ---------------- Phase D: attention, slot L then slot R ----------------
    with tc.tile_pool(name="attn", bufs=1) as attn, \
         tc.tile_pool(name="ptp", bufs=3) as ptp, \
         tc.tile_pool(name="aev", bufs=2) as aev, \
         tc.tile_pool(name="psO", bufs=1, space="PSUM") as psO, \
         tc.tile_pool(name="psS", bufs=2, space="PSUM") as psS, \
         tc.tile_pool(name="psL", bufs=1, space="PSUM") as psL:
        for slot, (recv_q, send_o) in enumerate(((recv_qa, send_oa),
                                                 (recv_qb, send_ob))):
            qTc = attn.tile([P, NTL * P], F16, tag="qTc", bufs=2)
            kTc = attn.tile([P, NTL * P], F16, tag="kTc", bufs=2)
            Vc = attn.tile([P, NTL, P], F16, tag="Vc", bufs=2)

            for r in range(4):
                tb = 4 * r * P
                nl = QTILES[r] * P
                nc.sync.dma_start(out=qTc[:, tb:tb + nl],
                                  in_=recv_q[r][:, 0:nl])
                nc.sync.dma_start(out=kTc[:, tb:tb + nl],
                                  in_=recv_q[r + 4][:, 0:nl])
                nc.sync.dma_start(
                    out=Vc[:, 4 * r:4 * r + QTILES[r], :],
                    in_=recv_v[4 * slot + r:4 * slot + r + 1, 0:nl * P]
                    .rearrange("one (a p d) -> (one p) a d", p=P, d=P))

            for (t0, t1) in GROUPS:
                ng = (t1 - t0 + 1) * P
                oT_ps = psO.tile([P, ng], F32, tag="oT")
                l_ps = psL.tile([1, ng], F32, tag="l")
                for kt in range(t1 + 1):
                    c0 = max(t0, kt)
                    off = (c0 - t0) * P
                    n = (t1 - c0 + 1) * P
                    sT_ps = psS.tile([P, n], F32, tag="sT")
                    for (ja, jn) in _chunks(n):
                        nc.tensor.matmul(sT_ps[:, ja:ja + jn],
                                         kTc[:, kt * P:(kt + 1) * P],
                                         qTc[:, c0 * P + ja:c0 * P + ja + jn],
                                         start=True, stop=True)
                    PT = ptp.tile([P, n], F16, tag="PT")
                    nc.scalar.activation(PT, sT_ps, AF.Exp, scale=SCALE,
                                         bias=neg4)
                    if kt == 16:
                        # zero pad-key rows 32..128 (a vector op with base
                        # partition 32 may span at most 32 rows)
                        nc.vector.tensor_scalar_mul(PT[32:64, :], PT[32:64, :], 0.0)
                        nc.vector.tensor_scalar_mul(PT[64:P, :], PT[64:P, :], 0.0)
                    # accumulation groups are per PSUM bank: a bank's last
                    # write happens at kt == its highest column tile
                    for (ja, jn) in _bank_chunks(off, n):
                        bank = ja // 512
                        fin = (kt == min(t1, t0 + 4 * bank + 3))
                        nc.tensor.matmul(oT_ps[:, ja:ja + jn],
                                         Vc[:, kt, :], PT[:, ja - off:ja - off + jn],
                                         start=(kt == 0), stop=fin)
                        nc.tensor.matmul(l_ps[:, ja:ja + jn],
                                         ones_col, PT[:, ja - off:ja - off + jn],
                                         start=(kt == 0), stop=fin)
                # evict group results, slicing into destination token blocks
                oT_sb = aev.tile([P, ng], F16, tag="oT_sb")
                nc.vector.tensor_copy(oT_sb, oT_ps)
                l_sb = aev.tile([1, ng], F16, tag="l_sb")
                nc.vector.tensor_copy(l_sb, l_ps)
                g0 = t0 * P
                for j in range(4):
                    a = max(g0, j * TOK_BLK)
                    b = min(g0 + ng, (j + 1) * TOK_BLK)
                    if a >= b:
                        continue
                    for dd in (j, j + 4):
                        nc.sync.dma_start(
                            out=send_o[dd, 0:P, a - j * TOK_BLK:b - j * TOK_BLK],
                            in_=oT_sb[:, a - g0:b - g0])
                        nc.sync.dma_start(
                            out=send_o[dd, P:P + 1,
                                       a - j * TOK_BLK:b - j * TOK_BLK],
                            in_=l_sb[:, a - g0:b - g0])
            if slot == 0:
                nc.gpsimd.collective_compute(
                    "AllToAll", ALU.bypass,
                    replica_groups=[list(range(N_CORES))],
                    ins=[send_oa.opt()], outs=[recv_oa.opt()])
        nc.gpsimd.collective_compute(
            "AllToAll", ALU.bypass, replica_groups=[list(range(N_CORES))],
            ins=[send_ob.opt()], outs=[recv_ob.opt()])

    # ---------------- Phase E: output projection (two passes) ----------------
    def head_src(h):
        # (rank, which recv buffer): L heads 0-5, R heads 6-11
        return (HSRC[h], recv_oa) if h < 6 else (HSRC[h - 6], recv_ob)

    with tc.tile_pool(name="oproj", bufs=1) as op, \
         tc.tile_pool(name="owork", bufs=3) as ow, \
         tc.tile_pool(name="psP", bufs=2, space="PSUM") as psP:

        oT_asm = op.tile([P, NH, TOK_BLK], F16)
        o_part = op.tile([P, 6, TOK_BLK], F32)

        def load_heads(h0):
            l16 = ow.tile([6, TOK_BLK], F16, tag="l16")
            for i in range(6):
                rk, rbuf = head_src(h0 + i)
                nc.sync.dma_start(out=l16[i:i + 1, :], in_=rbuf[rk, P:P + 1, :])
            l32 = ow.tile([6, TOK_BLK], F32, tag="l32")
            nc.vector.tensor_copy(l32, l16)
            rl32 = ow.tile([6, TOK_BLK], F32, tag="rl32")
            nc.vector.reciprocal(rl32, l32)
            rl16 = ow.tile([6, TOK_BLK], F16, tag="rl16")
            nc.vector.tensor_copy(rl16, rl32)
            nc.sync.dma_start(out=rl_dram[h0:h0 + 6, :], in_=rl16)
            for i in range(6):
                h = h0 + i
                rk, rbuf = head_src(h)
                oTh = ow.tile([P, TOK_BLK], F16, tag="oTh")
                nc.sync.dma_start(out=oTh, in_=rbuf[rk, 0:P, :])
                rlb = ow.tile([P, TOK_BLK], F16, tag="rlb")
                rl_bc = bass.AP(tensor=rl_dram.tensor,
                                offset=rl_dram.offset + h * TOK_BLK,
                                ap=[[0, P], [1, TOK_BLK]])
                nc.sync.dma_start(out=rlb, in_=rl_bc)
                nc.vector.tensor_tensor(oT_asm[:, h, :], oTh, rlb, ALU.mult)

        # pass 1: L heads (0-5), overlapped with the in-flight A2A of slot R
        load_heads(0)
        for m in range(6):
            ps = psP.tile([P, TOK_BLK], F32, tag="psP")
            for (ja, jn) in _chunks(TOK_BLK):
                for k in range(6):
                    nc.tensor.matmul(ps[:, ja:ja + jn],
                                     wo_sb[:, k, m * P:(m + 1) * P],
                                     oT_asm[:, k, ja:ja + jn],
                                     start=(k == 0), stop=(k == 5))
            nc.vector.tensor_copy(o_part[:, m, :], ps)

        # pass 2: R heads (6-11), then add the partial and emit
        load_heads(6)
        for m in range(6):
            ps = psP.tile([P, TOK_BLK], F32, tag="psP")
            for (ja, jn) in _chunks(TOK_BLK):
                for k in range(6, 12):
                    nc.tensor.matmul(ps[:, ja:ja + jn],
                                     wo_sb[:, k, m * P:(m + 1) * P],
                                     oT_asm[:, k, ja:ja + jn],
                                     start=(k == 6), stop=(k == 11))
            oev = ow.tile([P, TOK_BLK], F32, tag="oev")
            nc.vector.tensor_tensor(oev, ps, o_part[:, m, :], ALU.add)
            nc.sync.dma_start(out=outT[m * P:(m + 1) * P, :], in_=oev)


# ======================= host side =======================

def _expected_mask():
    blk = np.arange(SPAD) // P
    return (blk[:, None] >= blk[None, :]) & (np.arange(SPAD)[None, :] < S)


def _host_prep(x, freqs, wq, wk, wv, wo):
    """Build the 8 per-core input maps (fp16, x pre-transposed)."""
    x_pad = np.zeros((SPAD, DIM), np.float32)
    x_pad[:S] = x[0]

    # rope angle table (pure gather from freqs)
    t = np.arange(S)
    fi = t // (H_ * W_)
    hi = (t % (H_ * W_)) // W_
    wi = t % W_
    ang = np.zeros((SPAD, 64), np.float64)
    ang[:S, 0:22] = freqs[fi, 0:22]
    ang[:S, 22:43] = freqs[hi, 22:43]
    ang[:S, 43:64] = freqs[wi, 43:64]
    cos_all = np.cos(ang).astype(np.float16)
    sin_all = np.sin(ang).astype(np.float16)

    wq16, wk16 = wq.astype(np.float16), wk.astype(np.float16)
    wv16, wo16 = wv.astype(np.float16), wo.astype(np.float16)

    in_maps = []
    for c in range(N_CORES):
        qr = c % 4
        x_my = np.zeros((TQ, DIM), np.float32)
        x_my[:QLEN[qr]] = x_pad[QSTART[qr]:QSTART[qr] + QLEN[qr]]
        xT16 = np.ascontiguousarray(x_my.T.astype(np.float16))

        cos_my = np.zeros((TQ, 64), np.float16)
        sin_my = np.zeros((TQ, 64), np.float16)
        cos_my[:QLEN[qr]] = cos_all[QSTART[qr]:QSTART[qr] + QLEN[qr]]
        sin_my[:QLEN[qr]] = sin_all[QSTART[qr]:QSTART[qr] + QLEN[qr]]

        in_maps.append({
            "xT_my": xT16,
            "w_qk": np.ascontiguousarray(wq16 if c < 4 else wk16),
            "wv_half": np.ascontiguousarray(
                wv16[:, :VHALF] if c < 4 else wv16[:, VHALF:]),
            "wo_slice": np.ascontiguousarray(
                wo16[:, (c // 4) * NHALF:(c // 4 + 1) * NHALF]),
            "cos_in": cos_my,
            "sin_in": sin_my,
        })
    return in_maps


def get_nc():
    global _CACHED_NC
    if _CACHED_NC is None:
        _CACHED_NC = build_nc()
    return _CACHED_NC


def kernel(x, freqs, wq, bq, wk, bk, wv, bv, wo, bo, gq, gk,
           seq_lens, grid_sizes, mask, _run=None):
    x = np.asarray(x, np.float32)
    freqs = np.asarray(freqs, np.float64)
    wq, wk, wv, wo = (np.asarray(w, np.float32) for w in (wq, wk, wv, wo))

    assert x.shape == (1, S, DIM)
    assert int(np.asarray(seq_lens)[0]) == S, "teacher-forcing path not implemented"
    assert tuple(np.asarray(grid_sizes)[0]) == (F_, H_, W_)
    for b in (bq, bk, bv, bo):
        assert not np.any(np.asarray(b)), "nonzero bias not supported"
    for g in (gq, gk):
        assert np.all(np.asarray(g) == 1.0), "non-unit norm gain not supported"
    assert np.array_equal(np.asarray(mask), _expected_mask()), "unexpected mask"

    in_maps = _host_prep(x, freqs, wq, wk, wv, wo)

    if _run is None:
        from concourse.bass_utils import run_bass_kernel_spmd
        nc = get_nc()
        res = run_bass_kernel_spmd(nc, in_maps, list(range(N_CORES)))
        outs = [res.results[c]["outT"] for c in range(N_CORES)]
    else:
        outs = _run(in_maps)

    out = np.zeros((SPAD, DIM), np.float32)
    for c in range(N_CORES):
        r0 = (c % 4) * TOK_BLK
        n0 = (c // 4) * NHALF
        out[r0:r0 + TOK_BLK, n0:n0 + NHALF] = outs[c].T
    return out[:S][None]


# revision 16
# speedup vs baseline: 1.5985x; 1.0773x over previous
"""Trainium2 Bass kernel for CausalWanSelfAttention (block-causal attention with
RMSNorm'd+RoPE'd q/k), distributed over 8 NeuronCores via SPMD.

v2: fp16 datapath + static routing + pipelined collectives.

Sharding:
  - Token quarters (tiles 4/4/4/5 of 128): cores 0-3 compute the q projection
    (full 1536 dims) for their quarter + the LEFT v half (heads 0-5); cores
    4-7 compute k + the RIGHT v half (heads 6-11). x arrives pre-transposed
    (host) so no on-device transposes are needed for the projections.
  - Head slots: slot L of core d = head L[d] in [0..5], slot R = head
    L[d]+6. Heads 2,3,8,9 are computed twice (cores 6,7 duplicate; their
    output is discarded) which keeps the SPMD control flow uniform while
    making every A2A route static.
  - Collectives (all fp16): A2A_v (v columns per dest slot-head), A2A_qk
    split into L-rows and R-rows so slot-L attention can start one collective
    earlier, A2A_o per slot (oT + softmax denominator l per token block).
  - Attention: scores computed transposed (sT[keys,q] = kT.T @ qT) so P^T
    feeds the V-matmul directly; softmax denominator via ones-vector matmul;
    no running max (exp is offset by -4 to keep fp16 in range; the offset
    cancels in o/l).
  - Output projection: per-core [544-token x 768-out-dim] block, accumulated
    in two passes (L heads under the in-flight A2A_oR, then R heads).
"""
import math
import sys

sys.path.insert(0, "/opt/trn_rl_repo")

import numpy as np

import concourse.bass as bass
import concourse.tile as tile
from concourse import bacc, mybir
from concourse.masks import make_identity

F32 = mybir.dt.float32
F16 = mybir.dt.float16
AF = mybir.ActivationFunctionType
ALU = mybir.AluOpType

# problem constants (hardcoded per contract)
P = 128
DIM = 1536
NH = 12
HD = 128
S = 2080
SPAD = 2176
NTL = 17
F_, H_, W_ = 4, 20, 26
EPS = 1e-6
N_CORES = 8

QTILES = [4, 4, 4, 5]          # token tiles per quarter
QSTART = [0, 512, 1024, 1536]  # token start per quarter
QLEN = [512, 512, 512, 640]
TQ = 640                       # uniform (padded) tokens per core
NT = 5                         # uniform token tiles per core
VHALF = 768

# head-slot tables: slot L of core d = LTAB[d], slot R = LTAB[d] + 6.
LTAB = [0, 1, 2, 3, 4, 5, 2, 3]
# source core for head h at the o-projection (dup heads use the low core)
#   h < 6: core h's... head h is slot L of core d where LTAB[d]==h (lowest d)
HSRC = {h: LTAB.index(h) for h in range(6)}

TOK_BLK = 544                  # O-proj tokens per core
NHALF = 768                    # O-proj out-dims per core
OT_ROWS = 129                  # 128 oT rows + 1 l row
GROUPS = [(0, 3), (4, 7), (8, 11), (12, 16)]
SCALE = 1.0 / math.sqrt(HD)
EXP_OFF = -4.0                 # exp offset; cancels in o/l, keeps fp16 finite

_CACHED_NC = None


def _chunks(total, step=512):
    out = []
    a = 0
    while a < total:
        out.append((a, min(step, total - a)))
        a += step
    return out


def _bank_chunks(off, n, bank=512):
    """Split [off, off+n) at absolute multiples of `bank` (PSUM bank size)."""
    out = []
    a = off
    end = off + n
    while a < end:
        b = min(end, (a // bank + 1) * bank)
        out.append((a, b - a))
        a = b
    return out


def _pv(nc, oT_ps, Vc, t0, t1, kt, PT):
    """PV matmuls for one (group, kt): oT_ps[:, cols] += V_kt.T @ PT.

    PSUM accumulation groups are per bank: a bank's last write happens at
    kt == its highest column tile.
    """
    c0 = max(t0, kt)
    off = (c0 - t0) * P
    n = (t1 - c0 + 1) * P
    for (ja, jn) in _bank_chunks(off, n):
        bank = ja // 512
        fin = (kt == min(t1, t0 + 4 * bank + 3))
        nc.tensor.matmul(oT_ps[:, ja:ja + jn],
                         Vc[:, kt, :], PT[:, ja - off:ja - off + jn],
                         start=(kt == 0), stop=fin)


def build_nc():
    nc = bacc.Bacc("TRN2", target_bir_lowering=False, debug=False,
                   num_devices=N_CORES)

    xT_my = nc.dram_tensor("xT_my", [DIM, TQ], F16, kind="ExternalInput").ap()
    w_qk = nc.dram_tensor("w_qk", [DIM, DIM], F16, kind="ExternalInput").ap()
    wv_half = nc.dram_tensor("wv_half", [DIM, VHALF], F16,
                             kind="ExternalInput").ap()
    wo_slice = nc.dram_tensor("wo_slice", [DIM, NHALF], F16,
                              kind="ExternalInput").ap()
    cos_in = nc.dram_tensor("cos_in", [TQ, 64], F16, kind="ExternalInput").ap()
    sin_in = nc.dram_tensor("sin_in", [TQ, 64], F16, kind="ExternalInput").ap()
    outT = nc.dram_tensor("outT", [NHALF, TOK_BLK], F32,
                          kind="ExternalOutput").ap()

    with tile.TileContext(nc) as tc:
        _body(tc, xT_my, w_qk, wv_half, wo_slice, cos_in, sin_in, outT)
    nc.compile()
    return nc


def _body(tc, *args):
    from contextlib import ExitStack
    with ExitStack() as es:
        const = es.enter_context(tc.tile_pool(name="const", bufs=1))
        dram = es.enter_context(tc.tile_pool(name="dram", bufs=1, space="DRAM"))
        _body2(tc, const, dram, *args)


def _body2(tc, const, dram,
           xT_my, w_qk, wv_half, wo_slice, cos_in, sin_in, outT):
    nc = tc.nc

    ident = const.tile([P, P], F16)
    make_identity(nc, ident)
    ones_col = const.tile([P, 1], F16)
    nc.vector.memset(ones_col, 1.0)
    eps_t = const.tile([P, 1], F32)
    nc.vector.memset(eps_t, EPS)
    neg4 = const.tile([P, 1], F32)
    nc.vector.memset(neg4, EXP_OFF)

    cos_sb = const.tile([P, NT, 64], F16)
    sin_sb = const.tile([P, NT, 64], F16)
    nc.sync.dma_start(out=cos_sb, in_=cos_in.rearrange("(a p) c -> p a c", p=P))
    nc.sync.dma_start(out=sin_sb, in_=sin_in.rearrange("(a p) c -> p a c", p=P))

    # wqk + wo prefetch on the gpsimd DMA queue so they don't delay the
    # x^T / wv loads (sync queue) that gate the first matmul
    wqk_sb = const.tile([P, 12, DIM], F16)
    nc.gpsimd.dma_start(out=wqk_sb,
                        in_=w_qk.rearrange("(k p) d -> p k d", p=P))
    wo_sb = const.tile([P, 12, NHALF], F16)
    nc.gpsimd.dma_start(out=wo_sb,
                        in_=wo_slice.rearrange("(k p) d -> p k d", p=P))

    # collective buffers (all fp16)
    send_v = dram.tile([N_CORES, TQ * P], F16)
    recv_v = dram.tile([N_CORES, TQ * P], F16)
    send_qa = dram.tile([N_CORES, P, TQ], F16)
    recv_qa = dram.tile([N_CORES, P, TQ], F16)
    send_qb = dram.tile([N_CORES, P, TQ], F16)
    recv_qb = dram.tile([N_CORES, P, TQ], F16)
    send_oa = dram.tile([N_CORES, OT_ROWS, TOK_BLK], F16)
    recv_oa = dram.tile([N_CORES, OT_ROWS, TOK_BLK], F16)
    send_ob = dram.tile([N_CORES, OT_ROWS, TOK_BLK], F16)
    recv_ob = dram.tile([N_CORES, OT_ROWS, TOK_BLK], F16)
    rl_dram = dram.tile([NH, TOK_BLK], F16)

    # ------------- resident tensors (weights + x^T), prefetched -------------
    with tc.tile_pool(name="res", bufs=1) as res, \
         tc.tile_pool(name="work", bufs=4) as work, \
         tc.tile_pool(name="psA", bufs=3, space="PSUM") as psA, \
         tc.tile_pool(name="psT", bufs=2, space="PSUM") as psT:

        xT = res.tile([P, 12, TQ], F16)
        nc.sync.dma_start(out=xT,
                          in_=xT_my.rearrange("(k p) t -> p k t", p=P))
        wv_sb = res.tile([P, 12, VHALF], F16)
        nc.sync.dma_start(out=wv_sb,
                          in_=wv_half.rearrange("(k p) d -> p k d", p=P))

        # ---------------- Phase V: v projection + routing ----------------
        v_sb = res.tile([P, NT, VHALF], F16)
        for t in range(NT):
            for (n0, nn) in ((0, 512), (512, 256)):
                mm_ps = psA.tile([P, 512], F32, tag="mm")
                for k in range(12):
                    nc.tensor.matmul(mm_ps[:, 0:nn], xT[:, k, t * P:(t + 1) * P],
                                     wv_sb[:, k, n0:n0 + nn],
                                     start=(k == 0), stop=(k == 11))
                nc.vector.tensor_copy(v_sb[:, t, n0:n0 + nn], mm_ps[:, 0:nn])
        for d in range(N_CORES):
            c = LTAB[d]
            nc.sync.dma_start(
                out=send_v[d:d + 1, :].rearrange("one (a p v) -> (one p) a v",
                                                 p=P, v=P),
                in_=v_sb[:, :, c * P:(c + 1) * P])
        nc.gpsimd.collective_compute(
            "AllToAll", ALU.bypass, replica_groups=[list(range(N_CORES))],
            ins=[send_v.opt()], outs=[recv_v.opt()])

        # ---------------- Phase QK: projection + RMS + rope ----------------
        q_raw = res.tile([P, NT, DIM], F16)
        ssq = work.tile([P, NT, 3], F32, tag="ssq", bufs=1)
        for t in range(NT):
            for n in range(3):
                mm_ps = psA.tile([P, 512], F32, tag="mm")
                for k in range(12):
                    nc.tensor.matmul(mm_ps, xT[:, k, t * P:(t + 1) * P],
                                     wqk_sb[:, k, n * 512:(n + 1) * 512],
                                     start=(k == 0), stop=(k == 11))
                sq_scr = work.tile([P, 512], F16, tag="sq_scr")
                nc.scalar.activation(sq_scr, mm_ps, AF.Square,
                                     accum_out=ssq[:, t, n:n + 1])
                nc.vector.tensor_copy(q_raw[:, t, n * 512:(n + 1) * 512], mm_ps)

            # RMS + rope for tile t (in-place on q_raw)
            s01 = work.tile([P, 1], F32, tag="s01")
            nc.vector.tensor_tensor(s01, ssq[:, t, 0:1], ssq[:, t, 1:2], ALU.add)
            stot = work.tile([P, 1], F32, tag="stot")
            nc.vector.tensor_tensor(stot, s01, ssq[:, t, 2:3], ALU.add)
            sq_t = work.tile([P, 1], F32, tag="sq_t")
            nc.scalar.activation(sq_t, stot, AF.Sqrt, bias=eps_t,
                                 scale=1.0 / DIM)
            rsq = work.tile([P, 1], F32, tag="rsq")
            nc.vector.reciprocal(rsq, sq_t)
            crsq = work.tile([P, 64], F16, tag="crsq")
            srsq = work.tile([P, 64], F16, tag="srsq")
            nc.vector.tensor_scalar_mul(crsq, cos_sb[:, t, :], rsq)
            nc.vector.tensor_scalar_mul(srsq, sin_sb[:, t, :], rsq)
            cb = bass.AP(tensor=crsq.tensor, offset=crsq.offset,
                         ap=[crsq.ap[0], [0, NH], crsq.ap[1]])
            sbb = bass.AP(tensor=srsq.tensor, offset=srsq.offset,
                          ap=[srsq.ap[0], [0, NH], srsq.ap[1]])
            qh = q_raw[:, t, :].rearrange("p (h c two) -> p h c two", h=NH, two=2)
            qe = qh[:, :, :, 0]
            qo = qh[:, :, :, 1]
            tA = work.tile([P, NH, 64], F16, tag="tA")
            tB = work.tile([P, NH, 64], F16, tag="tB")
            tC = work.tile([P, NH, 64], F16, tag="tC")
            tD = work.tile([P, NH, 64], F16, tag="tD")
            nc.vector.tensor_tensor(tA, qe, cb, ALU.mult)
            nc.vector.tensor_tensor(tB, qo, sbb, ALU.mult)
            nc.vector.tensor_tensor(tC, qe, sbb, ALU.mult)
            nc.vector.tensor_tensor(tD, qo, cb, ALU.mult)
            nc.vector.tensor_tensor(qe, tA, tB, ALU.subtract)
            nc.vector.tensor_tensor(qo, tC, tD, ALU.add)

        # transpose roped q/k to d-major; L heads first so A2A_qa fires early
        qT_all = res.tile([P, 12, TQ], F16)
        for h0, send_q, recv_q in ((0, send_qa, recv_qa), (6, send_qb, recv_qb)):
            for h in range(h0, h0 + 6):
                for t in range(NT):
                    tp = psT.tile([P, P], F16, tag="tp")
                    nc.tensor.transpose(tp, q_raw[:, t, h * P:(h + 1) * P], ident)
                    nc.vector.tensor_copy(qT_all[:, h, t * P:(t + 1) * P], tp)
            for d in range(N_CORES):
                nc.sync.dma_start(out=send_q[d], in_=qT_all[:, LTAB[d] + h0, :])
            nc.gpsimd.collective_compute(
                "AllToAll", ALU.bypass, replica_groups=[list(range(N_CORES))],
                ins=[send_q.opt()], outs=[recv_q.opt()])

    # ---------------- Phase D: attention, slot L then slot R ----------------
    with tc.tile_pool(name="attn", bufs=1) as attn, \
         tc.tile_pool(name="ptp", bufs=3) as ptp, \
         tc.tile_pool(name="aev", bufs=2) as aev, \
         tc.tile_pool(name="psO", bufs=1, space="PSUM") as psO, \
         tc.tile_pool(name="psS", bufs=2, space="PSUM") as psS, \
         tc.tile_pool(name="psL", bufs=1, space="PSUM") as psL:
        for slot, (recv_q, send_o) in enumerate(((recv_qa, send_oa),
                                                 (recv_qb, send_ob))):
            qTc = attn.tile([P, NTL * P], F16, tag="qTc", bufs=2)
            kTc = attn.tile([P, NTL * P], F16, tag="kTc", bufs=2)
            Vc = attn.tile([P, NTL, P], F16, tag="Vc", bufs=2)

            for r in range(4):
                tb = 4 * r * P
                nl = QTILES[r] * P
                nc.sync.dma_start(out=qTc[:, tb:tb + nl],
                                  in_=recv_q[r][:, 0:nl])
                nc.sync.dma_start(out=kTc[:, tb:tb + nl],
                                  in_=recv_q[r + 4][:, 0:nl])
                nc.sync.dma_start(
                    out=Vc[:, 4 * r:4 * r + QTILES[r], :],
                    in_=recv_v[4 * slot + r:4 * slot + r + 1, 0:nl * P]
                    .rearrange("one (a p d) -> (one p) a d", p=P, d=P))

            for (t0, t1) in GROUPS:
                ng = (t1 - t0 + 1) * P
                oT_ps = psO.tile([P, ng], F32, tag="oT")
                l_ps = psL.tile([1, ng], F32, tag="l")
                acc = attn.tile([P, ng], F32, tag="acc", bufs=2)
                pend = None
                for kt in range(t1 + 1):
                    c0 = max(t0, kt)
                    off = (c0 - t0) * P
                    n = (t1 - c0 + 1) * P
                    sT_ps = psS.tile([P, n], F32, tag="sT")
                    for (ja, jn) in _chunks(n):
                        nc.tensor.matmul(sT_ps[:, ja:ja + jn],
                                         kTc[:, kt * P:(kt + 1) * P],
                                         qTc[:, c0 * P + ja:c0 * P + ja + jn],
                                         start=True, stop=True)
                    PT = ptp.tile([P, n], F16, tag="PT")
                    nc.scalar.activation(PT, sT_ps, AF.Exp, scale=SCALE,
                                         bias=neg4)
                    if kt == 16:
                        # zero pad-key rows 32..128 (a vector op with base
                        # partition 32 may span at most 32 rows)
                        nc.vector.tensor_scalar_mul(PT[32:64, :], PT[32:64, :], 0.0)
                        nc.vector.tensor_scalar_mul(PT[64:P, :], PT[64:P, :], 0.0)
                    # softmax denominator accumulates on the DVE (PE is the
                    # bottleneck here); kt == 0 always spans the full group
                    if kt == 0:
                        nc.vector.tensor_copy(acc, PT)
                    else:
                        nc.vector.tensor_tensor(acc[:, off:off + n],
                                                acc[:, off:off + n], PT, ALU.add)
                    # emit the previous kt's PV matmuls AFTER this kt's score
                    # matmuls: the PE never stalls waiting for exp(kt)
                    if pend is not None:
                        _pv(nc, oT_ps, Vc, t0, t1, *pend)
                    pend = (kt, PT)
                _pv(nc, oT_ps, Vc, t0, t1, *pend)
                # l = ones.T @ acc (single pass per group, PE cost ~ng)
                acc16 = aev.tile([P, ng], F16, tag="acc16")
                nc.vector.tensor_copy(acc16, acc)
                for (ja, jn) in _chunks(ng):
                    nc.tensor.matmul(l_ps[:, ja:ja + jn], ones_col,
                                     acc16[:, ja:ja + jn],
                                     start=True, stop=True)
                # evict group results, slicing into destination token blocks
                oT_sb = aev.tile([P, ng], F16, tag="oT_sb")
                nc.vector.tensor_copy(oT_sb, oT_ps)
                l_sb = aev.tile([1, ng], F16, tag="l_sb")
                nc.vector.tensor_copy(l_sb, l_ps)
                g0 = t0 * P
                for j in range(4):
                    a = max(g0, j * TOK_BLK)
                    b = min(g0 + ng, (j + 1) * TOK_BLK)
                    if a >= b:
                        continue
                    for dd in (j, j + 4):
                        nc.sync.dma_start(
                            out=send_o[dd, 0:P, a - j * TOK_BLK:b - j * TOK_BLK],
                            in_=oT_sb[:, a - g0:b - g0])
                        nc.sync.dma_start(
                            out=send_o[dd, P:P + 1,
                                       a - j * TOK_BLK:b - j * TOK_BLK],
                            in_=l_sb[:, a - g0:b - g0])
            if slot == 0:
                nc.gpsimd.collective_compute(
                    "AllToAll", ALU.bypass,
                    replica_groups=[list(range(N_CORES))],
                    ins=[send_oa.opt()], outs=[recv_oa.opt()])
        nc.gpsimd.collective_compute(
            "AllToAll", ALU.bypass, replica_groups=[list(range(N_CORES))],
            ins=[send_ob.opt()], outs=[recv_ob.opt()])

    # ---------------- Phase E: output projection (two passes) ----------------
    def head_src(h):
        # (rank, which recv buffer): L heads 0-5, R heads 6-11
        return (HSRC[h], recv_oa) if h < 6 else (HSRC[h - 6], recv_ob)

    with tc.tile_pool(name="oproj", bufs=1) as op, \
         tc.tile_pool(name="owork", bufs=3) as ow, \
         tc.tile_pool(name="psP", bufs=2, space="PSUM") as psP:

        oT_asm = op.tile([P, NH, TOK_BLK], F16)
        o_part = op.tile([P, 6, TOK_BLK], F32)

        def load_heads(h0):
            l16 = ow.tile([6, TOK_BLK], F16, tag="l16")
            for i in range(6):
                rk, rbuf = head_src(h0 + i)
                nc.sync.dma_start(out=l16[i:i + 1, :], in_=rbuf[rk, P:P + 1, :])
            l32 = ow.tile([6, TOK_BLK], F32, tag="l32")
            nc.vector.tensor_copy(l32, l16)
            rl32 = ow.tile([6, TOK_BLK], F32, tag="rl32")
            nc.vector.reciprocal(rl32, l32)
            rl16 = ow.tile([6, TOK_BLK], F16, tag="rl16")
            nc.vector.tensor_copy(rl16, rl32)
            nc.sync.dma_start(out=rl_dram[h0:h0 + 6, :], in_=rl16)
            # one broadcast DMA for all 6 heads' 1/l rows
            rlb = ow.tile([P, 6, TOK_BLK], F16, tag="rlb")
            rl_bc = bass.AP(tensor=rl_dram.tensor,
                            offset=rl_dram.offset + h0 * TOK_BLK,
                            ap=[[0, P], [1, 6 * TOK_BLK]])
            nc.sync.dma_start(out=rlb.rearrange("p a b -> p (a b)"), in_=rl_bc)
            for i in range(6):
                h = h0 + i
                rk, rbuf = head_src(h)
                oTh = ow.tile([P, TOK_BLK], F16, tag="oTh")
                nc.sync.dma_start(out=oTh, in_=rbuf[rk, 0:P, :])
                nc.vector.tensor_tensor(oT_asm[:, h, :], oTh, rlb[:, i, :],
                                        ALU.mult)

        # pass 1: L heads (0-5), overlapped with the in-flight A2A of slot R
        load_heads(0)
        for m in range(6):
            ps = psP.tile([P, TOK_BLK], F32, tag="psP")
            for (ja, jn) in _chunks(TOK_BLK):
                for k in range(6):
                    nc.tensor.matmul(ps[:, ja:ja + jn],
                                     wo_sb[:, k, m * P:(m + 1) * P],
                                     oT_asm[:, k, ja:ja + jn],
                                     start=(k == 0), stop=(k == 5))
            nc.vector.tensor_copy(o_part[:, m, :], ps)

        # pass 2: R heads (6-11), then add the partial and emit
        load_heads(6)
        for m in range(6):
            ps = psP.tile([P, TOK_BLK], F32, tag="psP")
            for (ja, jn) in _chunks(TOK_BLK):
                for k in range(6, 12):
                    nc.tensor.matmul(ps[:, ja:ja + jn],
                                     wo_sb[:, k, m * P:(m + 1) * P],
                                     oT_asm[:, k, ja:ja + jn],
                                     start=(k == 6), stop=(k == 11))
            oev = ow.tile([P, TOK_BLK], F32, tag="oev")
            nc.vector.tensor_tensor(oev, ps, o_part[:, m, :], ALU.add)
            nc.sync.dma_start(out=outT[m * P:(m + 1) * P, :], in_=oev)


# ======================= host side =======================

def _expected_mask():
    blk = np.arange(SPAD) // P
    return (blk[:, None] >= blk[None, :]) & (np.arange(SPAD)[None, :] < S)


def _host_prep(x, freqs, wq, wk, wv, wo):
    """Build the 8 per-core input maps (fp16, x pre-transposed)."""
    x_pad = np.zeros((SPAD, DIM), np.float32)
    x_pad[:S] = x[0]

    # rope angle table (pure gather from freqs)
    t = np.arange(S)
    fi = t // (H_ * W_)
    hi = (t % (H_ * W_)) // W_
    wi = t % W_
    ang = np.zeros((SPAD, 64), np.float64)
    ang[:S, 0:22] = freqs[fi, 0:22]
    ang[:S, 22:43] = freqs[hi, 22:43]
    ang[:S, 43:64] = freqs[wi, 43:64]
    cos_all = np.cos(ang).astype(np.float16)
    sin_all = np.sin(ang).astype(np.float16)

    wq16, wk16 = wq.astype(np.float16), wk.astype(np.float16)
    wv16, wo16 = wv.astype(np.float16), wo.astype(np.float16)

    in_maps = []
    for c in range(N_CORES):
        qr = c % 4
        x_my = np.zeros((TQ, DIM), np.float32)
        x_my[:QLEN[qr]] = x_pad[QSTART[qr]:QSTART[qr] + QLEN[qr]]
        xT16 = np.ascontiguousarray(x_my.T.astype(np.float16))

        cos_my = np.zeros((TQ, 64), np.float16)
        sin_my = np.zeros((TQ, 64), np.float16)
        cos_my[:QLEN[qr]] = cos_all[QSTART[qr]:QSTART[qr] + QLEN[qr]]
        sin_my[:QLEN[qr]] = sin_all[QSTART[qr]:QSTART[qr] + QLEN[qr]]

        in_maps.append({
            "xT_my": xT16,
            "w_qk": np.ascontiguousarray(wq16 if c < 4 else wk16),
            "wv_half": np.ascontiguousarray(
                wv16[:, :VHALF] if c < 4 else wv16[:, VHALF:]),
            "wo_slice": np.ascontiguousarray(
                wo16[:, (c // 4) * NHALF:(c // 4 + 1) * NHALF]),
            "cos_in": cos_my,
            "sin_in": sin_my,
        })
    return in_maps


def get_nc():
    global _CACHED_NC
    if _CACHED_NC is None:
        _CACHED_NC = build_nc()
    return _CACHED_NC


def kernel(x, freqs, wq, bq, wk, bk, wv, bv, wo, bo, gq, gk,
           seq_lens, grid_sizes, mask, _run=None):
    x = np.asarray(x, np.float32)
    freqs = np.asarray(freqs, np.float64)
    wq, wk, wv, wo = (np.asarray(w, np.float32) for w in (wq, wk, wv, wo))

    assert x.shape == (1, S, DIM)
    assert int(np.asarray(seq_lens)[0]) == S, "teacher-forcing path not implemented"
    assert tuple(np.asarray(grid_sizes)[0]) == (F_, H_, W_)
    for b in (bq, bk, bv, bo):
        assert not np.any(np.asarray(b)), "nonzero bias not supported"
    for g in (gq, gk):
        assert np.all(np.asarray(g) == 1.0), "non-unit norm gain not supported"
    assert np.array_equal(np.asarray(mask), _expected_mask()), "unexpected mask"

    in_maps = _host_prep(x, freqs, wq, wk, wv, wo)

    if _run is None:
        from concourse.bass_utils import run_bass_kernel_spmd
        nc = get_nc()
        res = run_bass_kernel_spmd(nc, in_maps, list(range(N_CORES)))
        outs = [res.results[c]["outT"] for c in range(N_CORES)]
    else:
        outs = _run(in_maps)

    out = np.zeros((SPAD, DIM), np.float32)
    for c in range(N_CORES):
        r0 = (c % 4) * TOK_BLK
        n0 = (c // 4) * NHALF
        out[r0:r0 + TOK_BLK, n0:n0 + NHALF] = outs[c].T
    return out[:S][None]
